# revision 25
# baseline (speedup 1.0000x reference)
"""DeformConvBlock Trainium2 kernel (data-parallel over batch across 8 cores).

Per-core (1 image, C=128, O=128, H=W=80, 3x3):
  1. offset = conv3x3(x, w_off) + b_off            (PE bf16 im2col GEMM)
  2. bilinear deform sampling via affine-basis identity:
       sample = P0[q] + dy*P1[q] + dx*P2[q] + dy*dx*P3[q],
     q = (floor(py), floor(px)) in an 8-padded image; P0..P3 = x and its
     v/h/cross shifted differences. One 1KB gather row per (tap,pixel),
     row layout [P0 P2 P1 P3] so the combine needs only 2 fused ops.
  3. P4 image built with DMA xbar transposes (no PE/ACT); dma_gather
     57.6K rows, round-robin over 4 SWDGE queues so descriptor generation
     runs on 4 Q7 core pairs concurrently.
  4. combine split across ACT (per-partition-scale muls) and DVE (2x-mode
     paired tensor_tensor adds + fused STTs); PE transpose; bf16 GEMM.
"""

import contextlib
import numpy as np
import ml_dtypes

import concourse.bass as bass
import concourse.tile as tile
from concourse import bacc, mybir
from concourse import bass_utils

F32 = mybir.dt.float32
BF16 = mybir.dt.bfloat16
I16 = mybir.dt.int16
I32 = mybir.dt.int32
A = mybir.AluOpType

N, C, O, H, W = 8, 128, 128, 80, 80
K = 9
PAD = 8
WP = H + 2 * PAD          # 96
QP = WP * WP              # 9216
HWi = H * W               # 6400
NT = HWi // 128           # 50 pixel tiles
NTT = NT * K              # 450 gather tiles
NJ = NTT * 128            # 57600 gather rows
CLAMP_MAX = float(WP - 2)
TCH = 2                   # pixel tiles per gather chunk
MT = 4                    # pixel tiles per maps iteration
NQ = 4                    # p4 build quarters
QPQ = QP // NQ            # 2304 padded pixels per quarter

# phase-3 tap routing: per chunk, tap units u=0..17; pairs routed via
# ACT-mul + paired DVE TT-add; the rest stay fully-fused on DVE.
ACT_PAIRS = [(0, 1), (2, 3), (4, 5), (6, 7), (8, 9), (10, 11)]
DVE_TAPS = [12, 13, 14, 15, 16, 17]


def build_kernel(num_devices=N):
    nc = bacc.Bacc("TRN2", target_bir_lowering=False, debug=False,
                   num_devices=num_devices, num_swdge_queues=4)

    x_in = nc.dram_tensor("x", [C, HWi], F32, kind="ExternalInput").ap()
    w_off_t = nc.dram_tensor("w_off_t", [C, K * 18], BF16, kind="ExternalInput").ap()
    w_t = nc.dram_tensor("w_t", [C, K * O], BF16, kind="ExternalInput").ap()
    b_in = nc.dram_tensor("b", [O, 1], F32, kind="ExternalInput").ap()
    baseT_in = nc.dram_tensor("baseT", [C, NT * 18], F32, kind="ExternalInput").ap()
    ident_in = nc.dram_tensor("ident", [128, 128], F32, kind="ExternalInput").ap()

    y_out = nc.dram_tensor("y", [O, HWi], F32, kind="ExternalOutput").ap()
    p4_dram = nc.dram_tensor("p4_dram", [QP, 4 * C], BF16, kind="Internal").ap()
    idx_dram = nc.dram_tensor("idx_dram", [C, NTT], I16, kind="Internal").ap()

    with tile.TileContext(nc) as tc:
        with contextlib.ExitStack() as ctx:
            _body(ctx, tc, nc, x_in, w_off_t, w_t, b_in, baseT_in,
                  ident_in, y_out, p4_dram, idx_dram)
    nc.compile()
    return nc


def _body(ctx, tc, nc, x_in, w_off_t, w_t, b_in, baseT_in,
          ident_in, y_out, p4_dram, idx_dram):
    const = ctx.enter_context(tc.tile_pool(name="const", bufs=1))
    pers = ctx.enter_context(tc.tile_pool(name="pers", bufs=1))

    # ---- constants ----
    ident = const.tile([128, 128], F32)
    nc.sync.dma_start(ident[:], ident_in)
    identb = const.tile([128, 128], BF16)
    nc.scalar.copy(identb[:], ident[:])
    bias = const.tile([O, 1], F32)
    nc.sync.dma_start(bias[:], b_in)
    baseT = const.tile([C, NT * 18], F32)
    nc.sync.dma_start(baseT[:], baseT_in)
    woff = const.tile([C, K * 18], BF16)
    nc.sync.dma_start(woff[:], w_off_t)
    wmat = const.tile([C, K * O], BF16)
    nc.sync.dma_start(wmat[:], w_t)

    # ---- persistent SBUF (live through phase 3) ----
    idxS = pers.tile([C, NTT], I16)
    ddS = pers.tile([C, 2 * NTT], F32)   # interleaved (dy, dx) per tap unit
    idxW = pers.tile([C, NJ // 16], I16)

    # ================= phase 1+2: load, offset conv, maps, planes, P4 =======
    with tc.tile_pool(name="ph1", bufs=1) as ph1, \
         tc.tile_pool(name="p4st", bufs=2) as p4st, \
         tc.tile_pool(name="mapsb", bufs=3) as sm, \
         tc.tile_pool(name="ps_off", bufs=3, space="PSUM") as ps_off, \
         tc.tile_pool(name="ps_mp", bufs=2, space="PSUM") as ps_mp:
        # x load: fp32 via HWDGE bands, cast to bf16 on ACT (idle at head)
        xb = ph1.tile([C, QP], BF16)
        nc.gpsimd.memset(xb[:], 0.0)
        xb3 = xb[:].rearrange("c (h w) -> c h w", h=WP)
        x3 = x_in.rearrange("c (h w) -> c h w", h=H)
        RB = 20
        for r0 in range(0, H, RB):
            xp = sm.tile([C, RB * W], F32, tag="xp")
            nc.sync.dma_start(xp[:], x_in[:, r0 * W:(r0 + RB) * W])
            nc.scalar.copy(xb3[:, PAD + r0:PAD + r0 + RB, PAD:PAD + W],
                           xp[:].rearrange("c (h w) -> c h w", h=RB))

        # bf16 difference planes (emitted early so the DMA xbar P4 build can
        # run while the conv owns the PE)
        d1 = ph1.tile([C, QP], BF16)
        nc.gpsimd.memset(d1[:, QP - WP:], 0.0)
        nc.vector.tensor_tensor(d1[:, :QP - WP], xb[:, WP:], xb[:, :QP - WP], op=A.subtract)
        d2 = ph1.tile([C, QP], BF16)
        nc.gpsimd.memset(d2[:, QP - 1:], 0.0)
        nc.vector.tensor_tensor(d2[:, :QP - 1], xb[:, 1:], xb[:, :QP - 1], op=A.subtract)
        d3 = ph1.tile([C, QP], BF16)
        nc.gpsimd.memset(d3[:, QP - WP:], 0.0)
        nc.vector.tensor_tensor(d3[:, :QP - WP], d2[:, WP:], d2[:, :QP - WP], op=A.subtract)

        # ---- P4 build via DMA xbar transposes; plane order [P0, P2, P1, P3].
        # xbars alternate between the two HWDGE rings (sync/scalar); the DRAM
        # writes go out on the SWDGE ring so all three overlap. ----
        planes = [xb, d2, d1, d3]
        for qi_, qr in enumerate(range(0, QP, QPQ)):
            stq = p4st.tile([128, QPQ // 128, 4 * C], BF16, tag="stq")
            for pi, pl in enumerate(planes):
                nc.sync.dma_start_transpose(stq[:, :, pi * C:(pi + 1) * C],
                                            pl[:, qr:qr + QPQ])
            dst = p4_dram[qr:qr + QPQ, :].rearrange("(blk p) c -> p blk c", p=128)
            nc.scalar.dma_start(dst, stq[:])

        # offset conv (bf16 in, fp32 accum), chunks of 6 output rows (N=480),
        # interleaved with the maps so DVE map work trails the conv chunk by
        # chunk instead of waiting for the whole conv
        off_sb = ph1.tile([18, HWi], F32)
        CH = 6

        def emit_conv_chunk(yc):
            rows = min(CH, H - yc)
            po = ps_off.tile([18, CH * W], F32, tag="po")
            for k in range(K):
                kh, kw = divmod(k, 3)
                rhs = xb3[:, (yc + kh - 1 + PAD):(yc + kh - 1 + PAD) + rows,
                          (kw - 1 + PAD):(kw - 1 + PAD) + W]
                nc.tensor.matmul(po[:, :rows * W],
                                 woff[:, k * 18:(k + 1) * 18], rhs,
                                 start=(k == 0), stop=(k == K - 1))
            nc.scalar.copy(off_sb[:, yc * W:(yc + rows) * W], po[:, :rows * W])

        def emit_maps_iter(t0):
            mt = min(MT, NT - t0)
            cols = mt * 18
            offT_ps = ps_mp.tile([128, MT * 18], F32, tag="offT")
            for i in range(mt):
                nc.tensor.transpose(offT_ps[:, i * 18:(i + 1) * 18],
                                    off_sb[:, (t0 + i) * 128:(t0 + i + 1) * 128],
                                    ident[0:18, 0:18])
            q = sm.tile([128, MT * 18], F32, tag="mq")
            nc.scalar.copy(q[:, :cols], offT_ps[:, :cols])
            nc.vector.tensor_tensor(q[:, :cols], q[:, :cols],
                                    baseT[:, t0 * 18:t0 * 18 + cols], op=A.add)
            nc.vector.tensor_scalar(q[:, :cols], q[:, :cols], CLAMP_MAX, 0.0,
                                    op0=A.min, op1=A.max)
            dd = ddS[:, 2 * K * t0:2 * K * t0 + cols]
            qi = sm.tile([128, MT * 18], I32, tag="mqi")
            nc.vector.tensor_copy(qi[:, :cols], q[:, :cols])          # rne
            qr = sm.tile([128, MT * 18], F32, tag="mqr")
            nc.vector.tensor_copy(qr[:, :cols], qi[:, :cols])
            m = sm.tile([128, MT * 18], F32, tag="mm")
            nc.vector.tensor_tensor(m[:, :cols], qr[:, :cols], q[:, :cols], op=A.is_gt)
            fl = sm.tile([128, MT * 18], F32, tag="mfl")
            nc.vector.tensor_tensor(fl[:, :cols], qr[:, :cols], m[:, :cols], op=A.subtract)
            nc.vector.tensor_tensor(dd, q[:, :cols], fl[:, :cols], op=A.subtract)
            fl2 = fl[:].rearrange("p (mk two) -> p mk two", two=2)
            nk = mt * K
            fidx = sm.tile([128, MT * K], F32, tag="mfi")
            nc.vector.scalar_tensor_tensor(fidx[:, :nk], fl2[:, :nk, 0], float(WP),
                                           fl2[:, :nk, 1], op0=A.mult, op1=A.add)
            nc.vector.tensor_copy(idxS[:, t0 * K:t0 * K + nk], fidx[:, :nk])

        def emit_wrap_half(h0):
            # idx wrap: j = T*128+pp -> wrapped[pp%16, 8T + pp//16]
            HT = NTT // 2
            nc.sync.dma_start(idx_dram[:, h0:h0 + HT], idxS[:, h0:h0 + HT])
            w1 = sm.tile([16, 8 * HT], I16, tag="w1")
            src2 = idx_dram[:, h0:h0 + HT].rearrange("(u r) t -> r u t", u=8)
            nc.sync.dma_start(w1[:].rearrange("r (u t) -> r u t", u=8), src2)
            w1v = w1[:].rearrange("r (u t) -> r t u", u=8)
            nc.vector.tensor_copy(
                idxW[0:16, 8 * h0:8 * (h0 + HT)].rearrange("r (t u) -> r t u", u=8), w1v)
            for g in range(1, 8):
                nc.sync.dma_start(idxW[16 * g:16 * (g + 1), 8 * h0:8 * (h0 + HT)],
                                  idxW[0:16, 8 * h0:8 * (h0 + HT)])

        # interleave: maps iter j needs off_sb pixels < 512*(j+1) = conv rows
        # < 6.4*(j+1); conv chunk c covers rows < 6*(c+1)
        maps_next = 0
        half0_done = False

        def maybe_wrap_half0():
            nonlocal half0_done
            if not half0_done and maps_next * K >= NTT // 2:
                emit_wrap_half(0)
                half0_done = True

        for ci_, yc in enumerate(range(0, H, CH)):
            emit_conv_chunk(yc)
            while maps_next < NT and 128 * (maps_next + MT) <= 480 * (ci_ + 1):
                emit_maps_iter(maps_next)
                maps_next = min(maps_next + MT, NT)
                maybe_wrap_half0()
        while maps_next < NT:
            emit_maps_iter(maps_next)
            maps_next = min(maps_next + MT, NT)
            maybe_wrap_half0()
        emit_wrap_half(NTT // 2)

    # ================= phase 3: gather + combine + GEMM =================
    with tc.tile_pool(name="gpool", bufs=5) as gpool, \
         tc.tile_pool(name="spool", bufs=8) as spool, \
         tc.tile_pool(name="vpool", bufs=4) as vpool, \
         tc.tile_pool(name="opool", bufs=3) as opool, \
         tc.tile_pool(name="ps_out", bufs=2, space="PSUM") as ps_out, \
         tc.tile_pool(name="ps_tp3", bufs=4, space="PSUM") as ps_tp:
        for ci, tc0 in enumerate(range(0, NT, TCH)):
            nidx = TCH * K * 128
            gt = gpool.tile([128, TCH * K, 4 * C], BF16, tag="gather")
            c0 = tc0 * K * 8
            nc.gpsimd.dma_gather(gt[:, :, :], p4_dram,
                                 idxW[:, c0:c0 + nidx // 16],
                                 num_idxs=nidx, num_idxs_reg=nidx, elem_size=4 * C,
                                 single_packet=False, queue_num=ci % 4)
            T0 = tc0 * K

            # stage 1: s12[u] = [P0+dy*P1 | P2+dy*P3] for all 18 tap units
            s12 = {}
            for u0, u1 in ACT_PAIRS:
                m12 = spool.tile([128, 2, 2 * C], BF16, tag="m12")
                for i, u in ((0, u0), (1, u1)):
                    nc.scalar.mul(m12[:, i, :], gt[:, u, 2 * C:4 * C],
                                  mul=ddS[:, 2 * (T0 + u):2 * (T0 + u) + 1])
                sx = spool.tile([128, 2, 2 * C], BF16, tag="s12x2")
                nc.vector.tensor_tensor(sx[:], m12[:], gt[:, u0:u1 + 1, 0:2 * C],
                                        op=A.add)
                s12[u0] = sx[:, 0, :]
                s12[u1] = sx[:, 1, :]
            for u in DVE_TAPS:
                sx = spool.tile([128, 2 * C], BF16, tag="s12")
                nc.vector.scalar_tensor_tensor(sx[:], gt[:, u, 2 * C:4 * C],
                                               ddS[:, 2 * (T0 + u):2 * (T0 + u) + 1],
                                               gt[:, u, 0:2 * C],
                                               op0=A.mult, op1=A.add)
                s12[u] = sx[:]

            # stage 2: v = s1 + dx*s2 (DVE), PE transpose, batched ACT copy
            vT = vpool.tile([C, TCH * K, 128], BF16, tag="vT")
            for g0 in range(0, 18, 4):
                g1 = min(g0 + 4, 18)
                tpp = ps_tp.tile([C, 4 * 128], BF16, tag="tpp")
                for u in range(g0, g1):
                    sx = s12[u]
                    v = spool.tile([128, C], BF16, tag="v")
                    nc.vector.scalar_tensor_tensor(
                        v[:], sx[:, C:2 * C],
                        ddS[:, 2 * (T0 + u) + 1:2 * (T0 + u) + 2],
                        sx[:, 0:C], op0=A.mult, op1=A.add)
                    nc.tensor.transpose(tpp[:, (u - g0) * 128:(u - g0 + 1) * 128],
                                        v[:], identb[:])
                nc.scalar.copy(vT[:, g0:g1, :], tpp[:, :(g1 - g0) * 128])

            out_ps = ps_out.tile([O, TCH * 128], F32, tag="ops")
            vT4 = vT[:].rearrange("c (t k) p -> c t k p", k=K)
            for k in range(K):
                nc.tensor.matmul(out_ps[:], wmat[:, k * O:(k + 1) * O],
                                 vT4[:, :, k, :],
                                 start=(k == 0), stop=(k == K - 1))
            ot = opool.tile([O, TCH * 128], F32, tag="ot")
            nc.vector.tensor_scalar_add(ot[:], out_ps[:], bias[:])
            nc.sync.dma_start(y_out[:, tc0 * 128:(tc0 + TCH) * 128], ot[:])


# ================= host side =================

def _prep_inputs(x, w_off, b_off, w, b):
    # [C, K*18]: col k*18+e = w_off[e, c, k]
    wofft = np.ascontiguousarray(
        w_off.reshape(18, C, K).transpose(1, 2, 0).reshape(C, K * 18)).astype(ml_dtypes.bfloat16)
    wt = np.ascontiguousarray(
        w.reshape(O, C, K).transpose(1, 2, 0).reshape(C, K * O)).astype(ml_dtypes.bfloat16)
    p = np.arange(HWi)
    py, px = p // W, p % W
    kh = np.arange(K) // 3 - 1
    kw = np.arange(K) % 3 - 1
    base = np.zeros((HWi, 18), np.float32)
    base[:, 0::2] = py[:, None] + kh[None, :] + PAD
    base[:, 1::2] = px[:, None] + kw[None, :] + PAD
    base += b_off.reshape(1, 18)
    baseT = np.ascontiguousarray(
        base.reshape(NT, 128, 18).transpose(1, 0, 2).reshape(128, NT * 18))
    ident = np.eye(128, dtype=np.float32)
    shared = {
        "w_off_t": wofft,
        "w_t": wt,
        "b": np.ascontiguousarray(b.reshape(O, 1)).astype(np.float32),
        "baseT": baseT,
        "ident": ident,
    }
    return [dict(shared, x=np.ascontiguousarray(x[n].reshape(C, HWi)).astype(np.float32))
            for n in range(x.shape[0])]


_CACHED = {}


def _get_nc(num_devices=N):
    key = num_devices
    if key not in _CACHED:
        _CACHED[key] = build_kernel(num_devices=num_devices)
    return _CACHED[key]


def kernel(x, w_off, b_off, w, b):
    x = np.asarray(x, np.float32)
    nc = _get_nc()
    core_ins = _prep_inputs(x, np.asarray(w_off, np.float32),
                            np.asarray(b_off, np.float32),
                            np.asarray(w, np.float32), np.asarray(b, np.float32))
    res = bass_utils.run_bass_kernel_spmd(nc, core_ins, core_ids=list(range(N)))
    return np.stack([res.results[n]["y"].reshape(O, H, W) for n in range(N)]).astype(np.float32)
